# revision 16
# baseline (speedup 1.0000x reference)
"""Multi-head attention (dense_transformer) on 8 TRN2 NeuronCores.

Sharding: data-parallel over batch (2) x tensor-parallel over head groups
(16 heads -> 4 groups of 4). Core (b, g) computes, for batch b:
  Q/K/V for its 4 heads (x @ w_qkv columns), per-head softmax(QK^T/sqrt(d))V,
  and the partial projection  attn_out_g @ w_proj[rows of g]  (+ b_proj/4).
The host sums the 4 partial projections per batch (the "all-reduce after
proj" of the sharding hint, done at gather time) and stacks the 2 batches.

All matmuls run on the PE array as float32r (full-rate fp32 path).
Softmax skips the max-subtraction (scores are ~N(0,1); exp is safe in fp32)
so the kernel needs no cross-partition max. The softmax denominator is
computed with a ones-matmul that also broadcasts l across 64 partitions,
packed into the spare column-half of the PV accumulation via PE col-tiling.
"""

import numpy as np

DIM = 1024
NUM_HEADS = 16
HEAD_DIM = 64
SCALE = HEAD_DIM ** -0.5
B = 2
N = 2048
NCORES = 8
HPG = 4               # heads per group (tensor-parallel degree 4)
GD = HPG * HEAD_DIM   # 256 dims per head group
CC = DIM // 128       # 8 contraction chunks over the model dim
TB = N // 128         # 16 token blocks
NQ = N // 512         # 4 query chunks
KB = N // 128         # 16 key blocks

_CACHE = {}


def _build_nc():
    from contextlib import ExitStack

    import concourse.tile as tile
    from concourse import bacc, mybir

    f32 = mybir.dt.float32
    f32r = mybir.dt.float32r
    EXP = mybir.ActivationFunctionType.Exp

    nc = bacc.Bacc("TRN2", target_bir_lowering=False, debug=False,
                   enable_asserts=False)

    # Matmul operands must be fp32r (e8m11 in the top 20 bits); the host
    # pre-rounds these inputs so the DMA can feed the PE directly.
    xt = nc.dram_tensor("xt", [DIM, N], f32r, kind="ExternalInput").ap()
    wq = nc.dram_tensor("wq", [DIM, GD], f32r, kind="ExternalInput").ap()
    wk = nc.dram_tensor("wk", [DIM, GD], f32r, kind="ExternalInput").ap()
    wv = nc.dram_tensor("wv", [DIM, GD], f32r, kind="ExternalInput").ap()
    wp = nc.dram_tensor("wp", [GD, DIM], f32r, kind="ExternalInput").ap()
    ones = nc.dram_tensor("ones", [1, 64], f32r, kind="ExternalInput").ap()
    out = nc.dram_tensor("out", [N, DIM], f32, kind="ExternalOutput").ap()

    with tile.TileContext(nc) as tc, ExitStack() as ctx:
        const = ctx.enter_context(tc.tile_pool(name="const", bufs=1))
        big = ctx.enter_context(tc.tile_pool(name="big", bufs=1))
        xts = ctx.enter_context(tc.tile_pool(name="xts", bufs=CC))
        pts = ctx.enter_context(tc.tile_pool(name="pts", bufs=3))
        outst = ctx.enter_context(tc.tile_pool(name="outst", bufs=2))
        small = ctx.enter_context(tc.tile_pool(name="small", bufs=2))
        ps_mm = ctx.enter_context(tc.tile_pool(name="ps_mm", bufs=2, space="PSUM"))
        ps_st = ctx.enter_context(tc.tile_pool(name="ps_st", bufs=2, space="PSUM"))
        ps_acc = ctx.enter_context(tc.tile_pool(name="ps_acc", bufs=2, space="PSUM"))

        # ---- resident inputs ------------------------------------------------
        xt_t = []
        for cc in range(CC):
            t = xts.tile([128, N], f32r, tag="xt")
            nc.sync.dma_start(t[:], xt[cc * 128:(cc + 1) * 128, :])
            xt_t.append(t)

        wq_sb = big.tile([128, CC, GD], f32r, tag="wq")
        nc.sync.dma_start(wq_sb[:], wq.rearrange("(cc p) d -> p cc d", p=128))
        wk_sb = big.tile([128, CC, GD], f32r, tag="wk")
        nc.sync.dma_start(wk_sb[:], wk.rearrange("(cc p) d -> p cc d", p=128))
        wv_sb = big.tile([128, CC, GD], f32r, tag="wv")
        nc.sync.dma_start(wv_sb[:], wv.rearrange("(cc p) d -> p cc d", p=128))
        wp_sb = big.tile([128, 2, DIM], f32r, tag="wp")
        nc.sync.dma_start(wp_sb[:], wp.rearrange("(dc p) d -> p dc d", p=128))
        qt_sb = big.tile([128, 2, N], f32r, tag="qt")   # Q^T: [d, tok]
        kt_sb = big.tile([128, 2, N], f32r, tag="kt")   # K^T: [d, tok]
        # V stored per (token-block, head) as [V_h | ones] (128 cols): the PV
        # matmul uses the whole 128-col block as lhsT (M=128) so PSUM rows
        # 0-63 get O^T_h and rows 64-127 get the softmax denom replicated 64x.
        import concourse.bass as bass_mod
        v_sb = big.tile([128, TB, HPG, 128], f32r, tag="v")
        for tb in range(TB):
            nc.sync.dma_start(
                v_sb[:, tb, :, 64:128],
                bass_mod.AP(tensor=ones.tensor, offset=ones.offset,
                            ap=[[0, 128], [0, HPG], [1, 64]]))
        ot_sb = big.tile([128, 2, N], f32r, tag="ot")   # attn-out^T: [d, tok]

        # ---- phase 1: QKV ---------------------------------------------------
        # Q^T[d,tok] / K^T[d,tok]: lhsT = w chunk [c,d], rhs = x^T chunk [c,tok]
        for w_sb, dst in ((wq_sb, qt_sb), (wk_sb, kt_sb)):
            for mb in range(2):
                for nq in range(NQ):
                    ps = ps_mm.tile([128, 512], f32, tag="mm")
                    for cc in range(CC):
                        nc.tensor.matmul(
                            ps[:],
                            w_sb[:, cc, mb * 128:(mb + 1) * 128],
                            xt_t[cc][:, nq * 512:(nq + 1) * 512],
                            start=(cc == 0), stop=(cc == CC - 1),
                        )
                    nc.vector.tensor_copy(
                        dst[:, mb, nq * 512:(nq + 1) * 512], ps[:])
        # V[tok,d]: lhsT = x^T chunk [c,tok], rhs = w chunk [c,d]
        for tb in range(TB):
            ps = ps_mm.tile([128, 512], f32, tag="mm")
            for cc in range(CC):
                nc.tensor.matmul(
                    ps[:, 0:GD],
                    xt_t[cc][:, tb * 128:(tb + 1) * 128],
                    wv_sb[:, cc, :],
                    start=(cc == 0), stop=(cc == CC - 1),
                )
            nc.vector.tensor_copy(
                v_sb[:, tb, :, 0:64],
                ps[:, 0:GD].rearrange("p (h d) -> p h d", h=HPG))

        # ---- phase 2: attention --------------------------------------------
        # Per head h: S^T[k,q] = (K^T_h)^T-stationary trick:
        #   lhsT = K^T_h [d=64, k-block 128], rhs = Q^T_h [d=64, q 512]
        # P^T = exp(SCALE * S^T) on ACT; then col-tiled pair into one PSUM bank:
        #   rows 0-63:  O^T_h += V_h^T-free matmul (lhsT = V[k,dh], rhs = P^T)
        #   rows 64-127: l (softmax denom) replicated 64x (lhsT = ones[k,64])
        for h in range(HPG):
            po = 64 * (h % 2)
            dc = h // 2
            for nq in range(NQ):
                acc = ps_acc.tile([128, 512], f32, tag="acc")
                for kb in range(KB):
                    st = ps_st.tile([128, 512], f32, tag="st")
                    nc.tensor.matmul(
                        st[:],
                        kt_sb[po:po + 64, dc, kb * 128:(kb + 1) * 128],
                        qt_sb[po:po + 64, dc, nq * 512:(nq + 1) * 512],
                        start=True, stop=True,
                    )
                    pt = pts.tile([128, 512], f32r, tag="pt")
                    nc.scalar.activation(pt[:], st[:], EXP, scale=SCALE)
                    nc.tensor.matmul(
                        acc[:], v_sb[:, kb, h, :], pt[:],
                        start=(kb == 0), stop=(kb == KB - 1),
                    )
                rec = small.tile([64, 512], f32, tag="rec")
                nc.vector.reciprocal(rec[:], acc[64:128, :])
                nc.vector.tensor_mul(
                    ot_sb[po:po + 64, dc, nq * 512:(nq + 1) * 512],
                    acc[0:64, :], rec[:])

        # ---- phase 3: projection (partial over this head group's rows) -----
        for tb in range(TB):
            for nb in range(2):
                ps = ps_mm.tile([128, 512], f32, tag="mm")
                for dc in range(2):
                    nc.tensor.matmul(
                        ps[:],
                        ot_sb[:, dc, tb * 128:(tb + 1) * 128],
                        wp_sb[:, dc, nb * 512:(nb + 1) * 512],
                        start=(dc == 0), stop=(dc == 1),
                    )
                ob = outst.tile([128, 512], f32, tag="ob")
                nc.vector.tensor_copy(ob[:], ps[:])
                nc.sync.dma_start(
                    out[tb * 128:(tb + 1) * 128, nb * 512:(nb + 1) * 512], ob[:])

    nc.compile()
    return nc


def get_nc():
    if "nc" not in _CACHE:
        _CACHE["nc"] = _build_nc()
    return _CACHE["nc"]


def round_fp32r(a):
    """Round fp32 to fp32r (e8m11: 11-bit mantissa, low 12 bits zero), RNE."""
    u = np.ascontiguousarray(a, dtype=np.float32).view(np.uint32)
    rne = (u >> np.uint32(12)) & np.uint32(1)
    u = (u + np.uint32(0x7FF) + rne) & np.uint32(0xFFFFF000)
    return u.view(np.float32)


def make_in_maps(x, w_qkv, w_proj, b_proj):
    x = np.ascontiguousarray(np.asarray(x, dtype=np.float32))
    w_qkv = np.asarray(w_qkv, dtype=np.float32)
    w_proj = np.asarray(w_proj, dtype=np.float32)
    b_proj = np.asarray(b_proj, dtype=np.float32)

    wr = w_qkv.reshape(DIM, 3, NUM_HEADS, HEAD_DIM)
    xts = [round_fp32r(x[b].T) for b in range(B)]
    ones = np.ones((1, 64), dtype=np.float32)

    in_maps = []
    for core in range(NCORES):
        b, g = divmod(core, HPG)
        h0, h1 = HPG * g, HPG * (g + 1)
        in_maps.append({
            "xt": xts[b],
            "wq": round_fp32r(wr[:, 0, h0:h1, :].reshape(DIM, GD)),
            "wk": round_fp32r(wr[:, 1, h0:h1, :].reshape(DIM, GD)),
            "wv": round_fp32r(wr[:, 2, h0:h1, :].reshape(DIM, GD)),
            "wp": round_fp32r(w_proj[g * GD:(g + 1) * GD, :]),
            "ones": ones,
        })
    return in_maps


def gather_out(results, b_proj):
    parts = [r["out"] for r in results]
    b_proj = np.asarray(b_proj, dtype=np.float32)
    return np.stack(
        [sum(parts[b * HPG:(b + 1) * HPG][1:], parts[b * HPG].copy()) + b_proj
         for b in range(B)],
        axis=0,
    ).astype(np.float32)


def kernel(x, w_qkv, w_proj, b_proj):
    from concourse import bass_utils

    nc = get_nc()
    in_maps = make_in_maps(x, w_qkv, w_proj, b_proj)
    res = bass_utils.run_bass_kernel_spmd(nc, in_maps, core_ids=list(range(NCORES)))
    return gather_out(res.results, b_proj)


# revision 20
# speedup vs baseline: 1.1236x; 1.1236x over previous
"""Multi-head attention (dense_transformer) on 8 TRN2 NeuronCores.

Sharding: data-parallel over batch (2) x tensor-parallel over head groups
(16 heads -> 4 groups of 4). Core (b, g) computes, for batch b:
  Q/K/V for its 4 heads (x @ w_qkv columns), per-head softmax(QK^T/sqrt(d))V,
  and the partial projection  attn_out_g @ w_proj[rows of g]  (+ b_proj/4).
The host sums the 4 partial projections per batch (the "all-reduce after
proj" of the sharding hint, done at gather time) and stacks the 2 batches.

All matmuls run on the PE array as float32r (full-rate fp32 path).
Softmax skips the max-subtraction (scores are ~N(0,1); exp is safe in fp32)
so the kernel needs no cross-partition max. The softmax denominator is
computed with a ones-matmul that also broadcasts l across 64 partitions,
packed into the spare column-half of the PV accumulation via PE col-tiling.
"""

import numpy as np

DIM = 1024
NUM_HEADS = 16
HEAD_DIM = 64
SCALE = HEAD_DIM ** -0.5
B = 2
N = 2048
NCORES = 8
HPG = 4               # heads per group (tensor-parallel degree 4)
GD = HPG * HEAD_DIM   # 256 dims per head group
CC = DIM // 128       # 8 contraction chunks over the model dim
TB = N // 128         # 16 token blocks
NQ = N // 512         # 4 query chunks
KB = N // 128         # 16 key blocks

_CACHE = {}


def _build_nc():
    from contextlib import ExitStack

    import concourse.tile as tile
    from concourse import bacc, mybir

    f32 = mybir.dt.float32
    f32r = mybir.dt.float32r
    EXP = mybir.ActivationFunctionType.Exp

    nc = bacc.Bacc("TRN2", target_bir_lowering=False, debug=False,
                   enable_asserts=False)

    # Matmul operands must be fp32r (e8m11 in the top 20 bits); the host
    # pre-rounds these inputs so the DMA can feed the PE directly.
    xt = nc.dram_tensor("xt", [DIM, N], f32r, kind="ExternalInput").ap()
    wq = nc.dram_tensor("wq", [DIM, GD], f32r, kind="ExternalInput").ap()
    wk = nc.dram_tensor("wk", [DIM, GD], f32r, kind="ExternalInput").ap()
    wv = nc.dram_tensor("wv", [DIM, GD], f32r, kind="ExternalInput").ap()
    wp = nc.dram_tensor("wp", [GD, DIM], f32r, kind="ExternalInput").ap()
    ones = nc.dram_tensor("ones", [1, 64], f32r, kind="ExternalInput").ap()
    out = nc.dram_tensor("out", [N, DIM], f32, kind="ExternalOutput").ap()

    with tile.TileContext(nc) as tc, ExitStack() as ctx:
        big = ctx.enter_context(tc.tile_pool(name="big", bufs=1))
        xts = ctx.enter_context(tc.tile_pool(name="xts", bufs=CC))
        pts = ctx.enter_context(tc.tile_pool(name="pts", bufs=2))
        outst = ctx.enter_context(tc.tile_pool(name="outst", bufs=2))
        small = ctx.enter_context(tc.tile_pool(name="small", bufs=2))
        # All of PSUM as 4 slots of 2 banks each: stA/stB hold S^T tiles
        # (and QKV/proj matmul outputs), accA/accB hold the PV accumulators
        # (and the V accumulators in phase 1).
        psum = ctx.enter_context(tc.tile_pool(name="psum", bufs=1, space="PSUM"))

        # ---- resident inputs ------------------------------------------------
        # xt DMA'd in (cc, nq) chunks, nq-major, so the first QT matmul group
        # can start after ~2 MB instead of the full 8 MB.
        xt_t = [xts.tile([128, N], f32r, tag="xt", name=f"xt{cc}")
                for cc in range(CC)]
        for nq in range(NQ):
            for cc in range(CC):
                nc.sync.dma_start(
                    xt_t[cc][:, nq * 512:(nq + 1) * 512],
                    xt[cc * 128:(cc + 1) * 128, nq * 512:(nq + 1) * 512])

        wq_sb = big.tile([128, CC, GD], f32r, tag="wq")
        nc.sync.dma_start(wq_sb[:], wq.rearrange("(cc p) d -> p cc d", p=128))
        wk_sb = big.tile([128, CC, GD], f32r, tag="wk")
        nc.sync.dma_start(wk_sb[:], wk.rearrange("(cc p) d -> p cc d", p=128))
        wv_sb = big.tile([128, CC, GD], f32r, tag="wv")
        nc.sync.dma_start(wv_sb[:], wv.rearrange("(cc p) d -> p cc d", p=128))
        wp_sb = big.tile([128, 2, DIM], f32r, tag="wp")
        nc.sync.dma_start(wp_sb[:], wp.rearrange("(dc p) d -> p dc d", p=128))
        qt_sb = big.tile([128, 2, N], f32r, tag="qt")   # Q^T: [d, tok]
        kt_sb = big.tile([128, 2, N], f32r, tag="kt")   # K^T: [d, tok]
        # V stored per (token-block, head) as [V_h | ones] (128 cols): the PV
        # matmul uses the whole 128-col block as lhsT (M=128) so PSUM rows
        # 0-63 get O^T_h and rows 64-127 get the softmax denom replicated 64x.
        import concourse.bass as bass_mod
        v_sb = big.tile([128, TB, HPG, 128], f32r, tag="v")
        for tb in range(TB):
            nc.sync.dma_start(
                v_sb[:, tb, :, 64:128],
                bass_mod.AP(tensor=ones.tensor, offset=ones.offset,
                            ap=[[0, 128], [0, HPG], [1, 64]]))
        ot_sb = big.tile([128, 2, N], f32r, tag="ot")   # attn-out^T: [d, tok]

        # ---- phase 1: QKV ---------------------------------------------------
        # Q^T[d,tok] / K^T[d,tok]: lhsT = w chunk [c,d], rhs = x^T chunk [c,tok]
        qkv_i = 0
        for w_sb, dst in ((wk_sb, kt_sb), (wq_sb, qt_sb)):
            for mb in range(2):
                for nq in range(NQ):
                    ps = psum.tile([128, 512], f32, tag=("stA", "stB")[qkv_i % 2], name=f"qkps{qkv_i}")
                    qkv_i += 1
                    for cc in range(CC):
                        nc.tensor.matmul(
                            ps[:],
                            w_sb[:, cc, mb * 128:(mb + 1) * 128],
                            xt_t[cc][:, nq * 512:(nq + 1) * 512],
                            start=(cc == 0), stop=(cc == CC - 1),
                        )
                    nc.vector.tensor_copy(
                        dst[:, mb, nq * 512:(nq + 1) * 512], ps[:])
        # V[tok,d]: lhsT = x^T chunk [c,tok], rhs = w chunk [c,d]
        for tb in range(TB):
            ps = psum.tile([128, 512], f32, tag=("accA", "accB")[tb % 2], name=f"vps{tb}")
            for cc in range(CC):
                nc.tensor.matmul(
                    ps[:, 0:GD],
                    xt_t[cc][:, tb * 128:(tb + 1) * 128],
                    wv_sb[:, cc, :],
                    start=(cc == 0), stop=(cc == CC - 1),
                )
            nc.vector.tensor_copy(
                v_sb[:, tb, :, 0:64],
                ps[:, 0:GD].rearrange("p (h d) -> p h d", h=HPG))

        # ---- phase 2: attention --------------------------------------------
        # Head pairs (hA, hB) interleaved so the PE always has independent
        # matmul work while ACT runs the other head's exp — keeps the PE
        # HAM-warm (2.4 GHz). Per head: S^T block via lhsT = K^T_h [d=64,
        # k-block], rhs = Q^T_h; P^T = exp(SCALE*S^T) width-1024 on ACT;
        # PV with lhsT = [V_h | ones] gives O^T_h (rows 0-63) and the
        # softmax denominator replicated 64x (rows 64-127).
        NQ2 = N // 1024
        for hp in range(2):
            hA, hB = 2 * hp, 2 * hp + 1
            for nq in range(NQ2):
                accA = psum.tile([128, 1024], f32, tag="accA")
                accB = psum.tile([128, 1024], f32, tag="accB")
                for kb in range(KB):
                    stA = psum.tile([128, 1024], f32, tag="stA")
                    stB = psum.tile([128, 1024], f32, tag="stB")
                    for q2 in range(2):
                        qsl = slice((2 * nq + q2) * 512, (2 * nq + q2 + 1) * 512)
                        ssl = slice(q2 * 512, (q2 + 1) * 512)
                        nc.tensor.matmul(
                            stA[:, ssl],
                            kt_sb[0:64, hp, kb * 128:(kb + 1) * 128],
                            qt_sb[0:64, hp, qsl],
                            start=True, stop=True,
                        )
                        nc.tensor.matmul(
                            stB[:, ssl],
                            kt_sb[64:128, hp, kb * 128:(kb + 1) * 128],
                            qt_sb[64:128, hp, qsl],
                            start=True, stop=True,
                        )
                    ptA = pts.tile([128, 1024], f32r, tag="ptA")
                    nc.scalar.activation(ptA[:], stA[:], EXP, scale=SCALE)
                    ptB = pts.tile([128, 1024], f32r, tag="ptB")
                    nc.scalar.activation(ptB[:], stB[:], EXP, scale=SCALE)
                    for q2 in range(2):
                        ssl = slice(q2 * 512, (q2 + 1) * 512)
                        nc.tensor.matmul(
                            accA[:, ssl], v_sb[:, kb, hA, :], ptA[:, ssl],
                            start=(kb == 0), stop=(kb == KB - 1),
                        )
                        nc.tensor.matmul(
                            accB[:, ssl], v_sb[:, kb, hB, :], ptB[:, ssl],
                            start=(kb == 0), stop=(kb == KB - 1),
                        )
                qsl = slice(nq * 1024, (nq + 1) * 1024)
                recA = small.tile([64, 1024], f32, tag="rec")
                nc.vector.reciprocal(recA[:], accA[64:128, :])
                nc.vector.tensor_mul(ot_sb[0:64, hp, qsl], accA[0:64, :], recA[:])
                recB = small.tile([64, 1024], f32, tag="rec")
                nc.vector.reciprocal(recB[:], accB[64:128, :])
                nc.vector.tensor_mul(ot_sb[64:128, hp, qsl], accB[0:64, :], recB[:])

        # ---- phase 3: projection (partial over this head group's rows) -----
        for tb in range(TB):
            for nb in range(2):
                ps = psum.tile([128, 512], f32, tag=("stA", "stB")[nb], name=f"pjps{tb}_{nb}")
                for dc in range(2):
                    nc.tensor.matmul(
                        ps[:],
                        ot_sb[:, dc, tb * 128:(tb + 1) * 128],
                        wp_sb[:, dc, nb * 512:(nb + 1) * 512],
                        start=(dc == 0), stop=(dc == 1),
                    )
                ob = outst.tile([128, 512], f32, tag="ob")
                nc.vector.tensor_copy(ob[:], ps[:])
                nc.sync.dma_start(
                    out[tb * 128:(tb + 1) * 128, nb * 512:(nb + 1) * 512], ob[:])

    nc.compile()
    return nc


def get_nc():
    if "nc" not in _CACHE:
        _CACHE["nc"] = _build_nc()
    return _CACHE["nc"]


def round_fp32r(a):
    """Round fp32 to fp32r (e8m11: 11-bit mantissa, low 12 bits zero), RNE."""
    u = np.ascontiguousarray(a, dtype=np.float32).view(np.uint32)
    rne = (u >> np.uint32(12)) & np.uint32(1)
    u = (u + np.uint32(0x7FF) + rne) & np.uint32(0xFFFFF000)
    return u.view(np.float32)


def make_in_maps(x, w_qkv, w_proj, b_proj):
    x = np.ascontiguousarray(np.asarray(x, dtype=np.float32))
    w_qkv = np.asarray(w_qkv, dtype=np.float32)
    w_proj = np.asarray(w_proj, dtype=np.float32)
    b_proj = np.asarray(b_proj, dtype=np.float32)

    wr = w_qkv.reshape(DIM, 3, NUM_HEADS, HEAD_DIM)
    xts = [round_fp32r(x[b].T) for b in range(B)]
    ones = np.ones((1, 64), dtype=np.float32)

    in_maps = []
    for core in range(NCORES):
        b, g = divmod(core, HPG)
        h0, h1 = HPG * g, HPG * (g + 1)
        in_maps.append({
            "xt": xts[b],
            "wq": round_fp32r(wr[:, 0, h0:h1, :].reshape(DIM, GD)),
            "wk": round_fp32r(wr[:, 1, h0:h1, :].reshape(DIM, GD)),
            "wv": round_fp32r(wr[:, 2, h0:h1, :].reshape(DIM, GD)),
            "wp": round_fp32r(w_proj[g * GD:(g + 1) * GD, :]),
            "ones": ones,
        })
    return in_maps


def gather_out(results, b_proj):
    parts = [r["out"] for r in results]
    b_proj = np.asarray(b_proj, dtype=np.float32)
    return np.stack(
        [sum(parts[b * HPG:(b + 1) * HPG][1:], parts[b * HPG].copy()) + b_proj
         for b in range(B)],
        axis=0,
    ).astype(np.float32)


def kernel(x, w_qkv, w_proj, b_proj):
    from concourse import bass_utils

    nc = get_nc()
    in_maps = make_in_maps(x, w_qkv, w_proj, b_proj)
    res = bass_utils.run_bass_kernel_spmd(nc, in_maps, core_ids=list(range(NCORES)))
    return gather_out(res.results, b_proj)
